# revision 3
# baseline (speedup 1.0000x reference)
"""ChebConv SpMM kernel for 8 TRN2 NeuronCores — gather + matmul-reduce design.

Per step (x_{k+1} = 2 L x_k - x_{k-1}), each core owns Vc=V/8 dest rows:
- Edges sorted by (source half, dest-tile group of GT=6 tiles, source region,
  dest tile). One dma_gather per (half, group, region) pulls all 6 tiles'
  x[col] rows (bf16, 256B) into an SBUF ring; descriptor generation is
  load-balanced across the 4 SWDGE queues by slot count.
- Reduction y[d] = sum val_e x[col_e] runs on the PE: per 128-slot column a
  matmul with a stationary sel block (sel[slot, d] = val_e, bf16) accumulates
  into a PSUM tile [128 dests, 128 cw] per dest tile.
- A sweep (lo sources): stAx = 2*psA - x_{k-1} staged to SBUF bf16 (fused
  recurrence term). B sweep (hi sources): y = 2*psB + stAx finalized directly
  in bf16 on DVE and written to HBM; AllGathers (Shared outputs) rebuild the
  full x for the next step, overlapped with the sweeps.
- Projection: cheb chunks read back transposed via HWDGE X-bar DMA (bf16),
  matmuls with the weight stationary and N=512 moving produce out^T; host
  transposes. Bias added on DVE.
"""
import sys

sys.path.insert(0, "/opt/trn_rl_repo")

import numpy as np
import ml_dtypes

import concourse.bass as bass
import concourse.bacc as bacc
import concourse.mybir as mybir
import concourse.tile as tile
from concourse import bass_utils

F32 = mybir.dt.float32
BF16 = mybir.dt.bfloat16
I16 = mybir.dt.int16
NPBF16 = ml_dtypes.bfloat16

NQ = 4        # SWDGE queues used for gathers
GT = 6        # dest tiles per gather group
CALL_COL_CAP = 8   # max 128-slot columns per gather call (ring-desc limit)
RBUFS = 16
CH = 512      # projection chunk rows (one PSUM bank at fp32)


class Plan:
    pass


# ----------------------------------------------------------------------------
# Host-side preprocessing
# ----------------------------------------------------------------------------

def build_plan(lap_rows, lap_cols, lap_vals, V, n_cores=8):
    C = n_cores
    Vc = V // C
    HALF = Vc // 2
    BUFROWS = HALF * C
    RG = min(32768, BUFROWS)
    NREG = (BUFROWS + RG - 1) // RG
    NT = Vc // 128
    assert NT % GT == 0
    NG = NT // GT

    rows = np.asarray(lap_rows).astype(np.int64)
    cols = np.asarray(lap_cols).astype(np.int64)
    vals = np.asarray(lap_vals).astype(np.float32)

    core = rows // Vc
    d = rows % Vc
    t = d // 128
    dd = d % 128
    g = t // GT
    ti = t % GT
    cv = cols // Vc
    off = cols % Vc
    buf = off // HALF
    srow = cv * HALF + (off % HALF)
    reg = srow // RG
    gi = (srow % RG).astype(np.int16)

    gkey = (((core * 2 + buf) * NG + g) * NREG + reg) * GT + ti
    order = np.argsort(gkey, kind="stable")
    gkey_s = gkey[order]
    counts = np.bincount(gkey_s, minlength=C * 2 * NG * NREG * GT)
    cnt = counts.reshape(C, 2, NG, NREG, GT)
    ncall = cnt.max(axis=0)                     # [2, NG, NREG, GT]
    assert ncall.min() > 0, "empty (half, group, region, tile) segment"
    ncols = (ncall + 127) // 128                # [2, NG, NREG, GT]

    flatc = ncols.reshape(-1)
    col_off = np.zeros_like(flatc)
    col_off[:] = np.r_[0, np.cumsum(flatc)[:-1]]
    col_off = col_off.reshape(2, NG, NREG, GT)
    NCB = int(flatc.sum())
    idx_off = col_off * 128                     # slots, 128 per column
    IDXTOT = NCB * 128

    gstart = np.r_[0, np.cumsum(counts)[:-1]]
    within = np.arange(len(gkey_s)) - gstart[gkey_s]
    gi_s = gi[order]
    dd_s = dd[order]
    vals_s = vals[order]
    c_s = gkey_s // (2 * NG * NREG * GT)
    rem = gkey_s % (2 * NG * NREG * GT)
    b_s = rem // (NG * NREG * GT)
    rem = rem % (NG * NREG * GT)
    g_s = rem // (NREG * GT)
    rem = rem % (NREG * GT)
    r_s = rem // GT
    ti_s = rem % GT

    # pads must be valid rows; spread them across the region so they don't
    # serialize on one HBM line (sel rows are 0, data unused)
    spread = ((np.arange(IDXTOT, dtype=np.int64) * 7919) % RG).astype(np.int16)
    gidx = np.tile(spread[None, :], (C, 1))
    gidx[c_s, idx_off[b_s, g_s, r_s, ti_s] + within] = gi_s

    blk = col_off[b_s, g_s, r_s, ti_s] + within // 128
    seldd = np.zeros((C, 128, NCB), np.float32)   # dd per slot (bf16-exact)
    selvv = np.zeros((C, 128, NCB), np.float32)   # val per slot (pads 0)
    seldd[c_s, within % 128, blk] = dd_s
    selvv[c_s, within % 128, blk] = vals_s

    plan = Plan()
    plan.C, plan.Vc, plan.HALF, plan.BUFROWS = C, Vc, HALF, BUFROWS
    plan.RG, plan.NREG, plan.NT, plan.NG = RG, NREG, NT, NG
    plan.V = V
    plan.ncall, plan.ncols = ncall, ncols
    plan.idx_off, plan.col_off = idx_off, col_off
    plan.IDXTOT, plan.NCB = IDXTOT, NCB
    plan.gidx = gidx
    plan.seldd = seldd
    plan.selvv = selvv
    return plan


def host_prep(lap_rows, lap_cols, lap_vals, inputs, weight, bias, n_cores=8):
    B, V, FIN = inputs.shape
    K, _, FOUT = weight.shape
    CW = B * FIN
    plan = build_plan(lap_rows, lap_cols, lap_vals, V, n_cores)
    plan.B, plan.FIN, plan.FOUT, plan.K, plan.CW = B, FIN, FOUT, K, CW
    BFO = B * FOUT
    x0 = np.ascontiguousarray(
        inputs.transpose(1, 0, 2).reshape(V, CW)).astype(np.float32)
    # Reference pairs cheb col (fi, k) with weight.reshape(K*Fin, F)[fi*K + k]
    W_eff = np.asarray(weight, np.float32).reshape(K * FIN, FOUT) \
        .reshape(FIN, K, FOUT).transpose(1, 0, 2)
    Wblk = np.zeros((K, CW, BFO), np.float32)
    for k in range(K):
        for b in range(B):
            Wblk[k, b * FIN:(b + 1) * FIN, b * FOUT:(b + 1) * FOUT] = W_eff[k]
    bias_col = np.tile(np.asarray(bias, np.float32), B).reshape(BFO, 1)

    iota_row = np.tile(np.arange(128, dtype=np.float32)[None, :],
                       (128, 1)).astype(NPBF16)
    in_maps = []
    for c in range(n_cores):
        gi_w = plan.gidx[c].reshape(plan.IDXTOT // 16, 16).T
        gi_w = np.tile(gi_w, (8, 1)).copy()
        x0c = x0[c * plan.Vc:(c + 1) * plan.Vc]
        in_maps.append({
            "x0h": x0c.astype(NPBF16),
            "gidx": gi_w,
            "seldd": plan.seldd[c].astype(NPBF16),
            "selvv": plan.selvv[c].astype(NPBF16),
            "wblkh": Wblk.astype(NPBF16), "bias_col": bias_col,
            "iotab": iota_row,
        })
    return plan, in_maps


# ----------------------------------------------------------------------------
# Device program
# ----------------------------------------------------------------------------

def build_calls(plan):
    """Per (half, group, region): list of calls [(idx_off, ncols,
    [(ti, tcols, selcol0), ...])] with <= CALL_COL_CAP columns each."""
    NG, NREG = plan.NG, plan.NREG
    ncols, col_off = plan.ncols, plan.col_off
    calls = {}
    for b in range(2):
        for g in range(NG):
            for r in range(NREG):
                lst = []
                cur = None
                for ti in range(GT):
                    tc = int(ncols[b, g, r, ti])
                    c0 = int(col_off[b, g, r, ti])
                    if cur is None or cur["ncols"] + tc > CALL_COL_CAP:
                        cur = {"col0": c0, "ncols": 0, "tiles": []}
                        lst.append(cur)
                    cur["tiles"].append((ti, tc, c0))
                    cur["ncols"] += tc
                calls[(b, g, r)] = lst
    return calls


def build_program(plan, n_cores=8):
    C, Vc, HALF, BUFROWS = plan.C, plan.Vc, plan.HALF, plan.BUFROWS
    RG, NREG, NT, NG = plan.RG, plan.NREG, plan.NT, plan.NG
    K, B, FIN, FOUT, CW = plan.K, plan.B, plan.FIN, plan.FOUT, plan.CW
    KS = K - 1
    BFO = B * FOUT
    ncols, col_off = plan.ncols, plan.col_off
    IDXTOT, NCB = plan.IDXTOT, plan.NCB
    calls = build_calls(plan)

    gcols = ncols.sum(axis=(2, 3))               # [2, NG] sel cols per sweep
    GCOLS_MAX = int(gcols.max())
    AG_LO_G = NG // 2 - 1
    NCH = Vc // CH

    nc = bacc.Bacc("TRN2", target_bir_lowering=False, debug=False,
                   num_devices=n_cores, num_swdge_queues=NQ)
    x0h = nc.dram_tensor("x0h", [Vc, CW], BF16, kind="ExternalInput")
    gidx = nc.dram_tensor("gidx", [128, IDXTOT // 16], I16,
                          kind="ExternalInput")
    seldd = nc.dram_tensor("seldd", [128, NCB], BF16, kind="ExternalInput")
    selvv = nc.dram_tensor("selvv", [128, NCB], BF16, kind="ExternalInput")
    iotab = nc.dram_tensor("iotab", [128, 128], BF16, kind="ExternalInput")
    wblkh = nc.dram_tensor("wblkh", [K, CW, BFO], BF16, kind="ExternalInput")
    bias_col = nc.dram_tensor("bias_col", [BFO, 1], F32,
                              kind="ExternalInput")
    outT = nc.dram_tensor("outT", [BFO, Vc], F32, kind="ExternalOutput")

    x0hb = nc.dram_tensor("x0hb", [Vc, CW], BF16)
    selc = nc.dram_tensor("selc", [128, NCB * 128], BF16)
    xfa = [[nc.dram_tensor(f"xf{k}_{h}", [BUFROWS, CW], BF16,
                           addr_space="Shared")
            for h in range(2)] for k in range(KS)]
    ybh = [None] + [nc.dram_tensor(f"yh{k}", [Vc, CW], BF16)
                    for k in range(1, KS + 1)]

    qslots = [0] * NQ  # per-queue slot counters for balanced assignment

    def gen_sel(k, b, g, sel_tile, iot, ddp, vvp):
        """sel[:, j*128+d] = (iota == dd_j) * val_j, regenerated on DVE
        every step: ~2.4MB/step of dd/vv streams instead of 57MB/step of
        cached sel blocks (frees SDMA engines for the gather-ring drain)."""
        base = int(col_off[b, g, 0, 0])
        ncg = int(gcols[b, g])
        dd = ddp.tile([128, GCOLS_MAX], BF16, tag="dd")
        nc.scalar.dma_start(dd[:, :ncg], seldd[:, base:base + ncg])
        vv = vvp.tile([128, GCOLS_MAX], BF16, tag="vv")
        nc.scalar.dma_start(vv[:, :ncg], selvv[:, base:base + ncg])
        sel3 = sel_tile[:, :ncg * 128].rearrange("p (n c) -> p n c", n=ncg)
        nc.vector.tensor_tensor(
            sel3, dd[:, :ncg].unsqueeze(-1).broadcast_to([128, ncg, 128]),
            iot[:].unsqueeze(1).broadcast_to([128, ncg, 128]),
            mybir.AluOpType.is_equal)
        nc.vector.tensor_tensor(
            sel3, sel3,
            vv[:, :ncg].unsqueeze(-1).broadcast_to([128, ncg, 128]),
            mybir.AluOpType.mult)

    def emit_gathers(k, b, g, rg_pool, git):
        """Fire all of group (b, g)'s gathers into fresh ring tiles; returns
        per-ti list of (rg_tile, ring_col, sel_col) matmul operands."""
        base = int(col_off[b, g, 0, 0])
        per_ti = {ti: [] for ti in range(GT)}
        for r in range(NREG):
            for call in calls[(b, g, r)]:
                ncls = call["ncols"] * 128
                io = call["col0"] * 128
                rg = rg_pool.tile([128, CALL_COL_CAP, CW], BF16, tag="ring")
                q = min(range(NQ), key=lambda i: qslots[i])
                qslots[q] += call["ncols"]
                nc.gpsimd.dma_gather(
                    rg[:, :call["ncols"], :],
                    xfa[k][b][r * RG:min((r + 1) * RG, BUFROWS), :],
                    git[:, io // 16:(io + ncls) // 16],
                    ncls, ncls, CW, queue_num=q)
                cpos = 0
                for (ti, tc, c0) in call["tiles"]:
                    soff = c0 - base
                    for j in range(tc):
                        per_ti[ti].append((rg, cpos + j, soff + j))
                    cpos += tc
        return per_ti

    def emit_matmuls(per_ti, ti, sel_tile, ps):
        """One contiguous PSUM accumulation chain for dest tile ti."""
        ops = per_ti[ti]
        for i, (rg, rcol, scol) in enumerate(ops):
            nc.tensor.matmul(
                ps[:], sel_tile[:, scol * 128:(scol + 1) * 128],
                rg[:, rcol, :],
                start=(i == 0), stop=(i == len(ops) - 1))

    import contextlib
    with tile.TileContext(nc) as tc:
        with contextlib.ExitStack() as es:
            pool = lambda *a, **k: es.enter_context(tc.tile_pool(*a, **k))
            gip = pool(name="gi", bufs=1)
            cwp = pool(name="cw", bufs=1)
            stAp = pool(name="stA", bufs=1)
            ring = pool(name="ring", bufs=RBUFS)
            selp = pool(name="sel", bufs=3)
            ddp = pool(name="dd", bufs=3)
            vvp = pool(name="vv", bufs=3)
            xpp = pool(name="xp", bufs=2)
            sthp = pool(name="sth", bufs=3)
            chtp = pool(name="cht", bufs=10)
            ostp = pool(name="ost", bufs=3)
            psAp = pool(name="psA", bufs=2, space="PSUM")
            psBp = pool(name="psB", bufs=2, space="PSUM")
            poTp = pool(name="poT", bufs=4, space="PSUM")
            # prologue: x0 bounce + AllGather (bf16), constants, indices
            nc.sync.dma_start(x0hb[:], x0h[:])
            for h in range(2):
                nc.gpsimd.collective_compute(
                    "AllGather", mybir.AluOpType.bypass,
                    replica_groups=[list(range(n_cores))],
                    ins=[x0hb[h * HALF:(h + 1) * HALF, :].opt()],
                    outs=[xfa[0][h].ap().opt()])
            git = gip.tile([128, IDXTOT // 16], I16, tag="git")
            nc.sync.dma_start(git[:], gidx[:])
            wt = []
            for k in range(K):
                wtk = cwp.tile([CW, BFO], BF16, tag=f"w{k}")
                nc.sync.dma_start(wtk[:], wblkh[k, :, :])
                wt.append(wtk)
            btT = cwp.tile([128, 2], F32, tag="biasT")
            nc.sync.dma_start(btT[:],
                              bias_col[:].rearrange("(h p) o -> p (h o)", p=128))
            iot = cwp.tile([128, 128], BF16, tag="iota")
            nc.sync.dma_start(iot[:], iotab[:])
            stAx = stAp.tile([128, NT, 128], BF16, tag="stAx")

            for i in range(RBUFS):   # clear ring bufs once (stale-NaN guard)
                rz = ring.tile([128, CALL_COL_CAP, CW], BF16, tag="ring")
                nc.vector.memset(rz[:], 0.0)

            for k in range(KS):
                # ---- A sweep (lo half sources): stAx = 2*psA - x_{k-1} ----
                for g in range(NG):
                    sa = selp.tile([128, GCOLS_MAX * 128], BF16, tag="selA")
                    gen_sel(k, 0, g, sa, iot, ddp, vvp)
                    per_ti = emit_gathers(k, 0, g, ring, git)
                    r0 = g * GT * 128
                    rows = GT * 128
                    xp = None
                    if k > 0:
                        xsrc = x0h if k == 1 else ybh[k - 1]
                        xp = xpp.tile([128, GT, CW], BF16, tag="xp")
                        nc.sync.dma_start(
                            xp[:], xsrc[r0:r0 + rows, :].rearrange(
                                "(ts p) c -> p ts c", p=128))
                    for ti in range(GT):
                        ps = psAp.tile([128, 128], F32, tag="psA")
                        emit_matmuls(per_ti, ti, sa, ps)
                        if k == 0:
                            nc.vector.tensor_copy(stAx[:, g * GT + ti, :],
                                                  ps[:])
                        else:
                            nc.vector.scalar_tensor_tensor(
                                stAx[:, g * GT + ti, :], ps[:], 2.0,
                                xp[:, ti, :],
                                mybir.AluOpType.mult,
                                mybir.AluOpType.subtract)

                # ---- B sweep (hi half): y = 2*psB + stAx (bf16) ----
                for g in range(NG):
                    sb = selp.tile([128, GCOLS_MAX * 128], BF16, tag="selB")
                    gen_sel(k, 1, g, sb, iot, ddp, vvp)
                    per_ti = emit_gathers(k, 1, g, ring, git)
                    r0 = g * GT * 128
                    rows = GT * 128
                    sth = sthp.tile([128, GT, CW], BF16, tag="sth")
                    for ti in range(GT):
                        ps = psBp.tile([128, 128], F32, tag="psB")
                        emit_matmuls(per_ti, ti, sb, ps)
                        if k == 0:
                            nc.vector.tensor_tensor(
                                sth[:, ti, :], ps[:], stAx[:, g * GT + ti, :],
                                mybir.AluOpType.add)
                        else:
                            nc.vector.scalar_tensor_tensor(
                                sth[:, ti, :], ps[:], 2.0,
                                stAx[:, g * GT + ti, :],
                                mybir.AluOpType.mult,
                                mybir.AluOpType.add)
                    nc.sync.dma_start(
                        ybh[k + 1][r0:r0 + rows, :].rearrange(
                            "(ts p) c -> p ts c", p=128), sth[:])
                    if k < KS - 1:
                        if g == AG_LO_G:
                            nc.gpsimd.collective_compute(
                                "AllGather", mybir.AluOpType.bypass,
                                replica_groups=[list(range(n_cores))],
                                ins=[ybh[k + 1][0:HALF, :].opt()],
                                outs=[xfa[k + 1][0].ap().opt()])
                        elif g == NG - 1:
                            nc.gpsimd.collective_compute(
                                "AllGather", mybir.AluOpType.bypass,
                                replica_groups=[list(range(n_cores))],
                                ins=[ybh[k + 1][HALF:Vc, :].opt()],
                                outs=[xfa[k + 1][1].ap().opt()])

            # ---- projection: out^T[bfo, rows] = sum_k W_k^T cheb_k^T ----
            chsrc = [x0h] + [ybh[kk] for kk in range(1, K)]
            for c in range(NCH):
                chts = []
                for kk in range(K):
                    cht = chtp.tile([128, CH], BF16, tag="cht")
                    # all on nc.sync: same HWDGE ring as the ybh writes, so
                    # ring-FIFO order gives the RAW ordering on ybh[4]
                    nc.sync.dma_start(cht[:],
                                      chsrc[kk][c * CH:(c + 1) * CH, :],
                                      transpose=True)
                    chts.append(cht)
                for h in range(2):
                    po = poTp.tile([128, CH], F32, tag="poT")
                    for kk in range(K):
                        nc.tensor.matmul(
                            po[:], wt[kk][:, h * 128:(h + 1) * 128],
                            chts[kk][:],
                            start=(kk == 0), stop=(kk == K - 1))
                    ost = ostp.tile([128, CH], F32, tag="ost")
                    nc.vector.tensor_tensor(
                        ost[:], po[:],
                        btT[:, h:h + 1].broadcast_to([128, CH]),
                        mybir.AluOpType.add)
                    nc.sync.dma_start(
                        outT[h * 128:(h + 1) * 128, c * CH:(c + 1) * CH],
                        ost[:])
    nc.compile()
    return nc


def gather_out(plan, res, n_cores=8):
    outs = []
    B, FOUT = plan.B, plan.FOUT
    for c in range(n_cores):
        o = res.results[c]["outT"]                      # [BFO, Vc]
        outs.append(np.ascontiguousarray(o.T)
                    .reshape(plan.Vc, B, FOUT).transpose(1, 0, 2))
    return np.ascontiguousarray(np.concatenate(outs, axis=1)).astype(
        np.float32)


# ----------------------------------------------------------------------------
# Self-contained entry point
# ----------------------------------------------------------------------------

_CACHE = {}


def kernel(lap_rows, lap_cols, lap_vals, inputs, weight, bias):
    inputs = np.asarray(inputs, np.float32)
    weight = np.asarray(weight, np.float32)
    bias = np.asarray(bias, np.float32)
    B, V, FIN = inputs.shape
    K, _, FOUT = weight.shape
    n_cores = 8
    plan, in_maps = host_prep(lap_rows, lap_cols, lap_vals, inputs, weight,
                              bias, n_cores)
    key = ("v9", V, plan.IDXTOT, plan.NCB)
    if key not in _CACHE:
        _CACHE.clear()
        _CACHE[key] = build_program(plan, n_cores)
    nc = _CACHE[key]
    res = bass_utils.run_bass_kernel_spmd(nc, in_maps,
                                          core_ids=list(range(n_cores)))
    return gather_out(plan, res, n_cores)
